# revision 7
# baseline (speedup 1.0000x reference)
"""Trainium2 Bass kernel for nn_ConvolutionDMax (segment_reduce).

Position-major ("interleaved") layout: slots of equal size s are grouped in
subgroups of n slots; token j of slot i lives at column j*n + i. Conv taps
become matmuls shifted by k*n. Segment max then decomposes into contiguous
tensor_tensor MAX folds (DVE 2x fp16 mode) + one small strided reduce,
instead of a full-rate-limited tensor_reduce over every element (DVE
tensor_reduce runs at 1 elem/cycle regardless of dtype -- measured).

Per supertile (<=1024 PSUM cols, 3 streams):
  - PE: 6 accumulating bf16 matmuls (128x128x<=512).
  - w1/w2: ACT evicts PSUM -> SBUF fp16 with fused Relu+bias (max commutes),
    GPSIMD zeroes the ragged tail blocks (relu'd data >= 0 so zeros are
    neutral), DVE runs 2 contiguous fold halvings + strided reduce tail.
  - w0: DVE strided max reduce direct from PSUM (exact span, raw values).
Final: relu+bias on pooled0, linear via fp16 matmuls, tanh, DMA out.
"""

import os
from collections import defaultdict

import ml_dtypes
import numpy as np

N_CORES = 8
C = 128          # feature dim (partition dim everywhere)
ST = 1024        # supertile positions (2 PSUM banks, fp32)
CHUNK_STS = 4    # supertiles per DMA chunk
MM = 512         # max matmul free dim

_PROGRAM_CACHE = {}
LAST_RESULTS = None  # BassKernelResults of the most recent run (for test.py)


# --------------------------------------------------------------------------
# Layout planning (pure python/numpy, no device deps)
# --------------------------------------------------------------------------

class _Plan:
    __slots__ = (
        "template", "assign", "subgroups", "sts", "chunks", "L", "nslot",
        "nslot2", "max_clen", "halo",
    )


def _build_plan(sizes: np.ndarray) -> _Plan:
    """Template layout shared by all cores + per-core slot assignment.

    subgroups: list of (gbase, slot0, n, s, sp) -- n same-size slots laid out
    position-major in cols [gbase, gbase + n*sp).
    """
    by_size = defaultdict(list)
    for i, s in enumerate(sizes.tolist()):
        by_size[int(s)].append(i)

    template = []                      # slot -> segment size
    assign = [[] for _ in range(N_CORES)]  # core -> slot -> orig idx or -1
    class_counts = []                  # (s, m) in template order
    for s in sorted(by_size, reverse=True):
        idxs = by_size[s]
        m = -(-len(idxs) // N_CORES)
        class_counts.append((s, m))
        for j in range(m):
            template.append(s)
            for c in range(N_CORES):
                k = j * N_CORES + c
                assign[c].append(idxs[k] if k < len(idxs) else -1)

    # Subgroups: per class, chunks of up to nmax slots (nmax even).
    subgroups = []
    off = 0
    slot0 = 0
    for s, m in class_counts:
        sp = (s + 3) & ~3  # mult of 4: fold alignment + even everywhere
        nmax = ST // sp
        if nmax > 1:
            nmax &= ~1
        left = m
        while left > 0:
            n = min(nmax, left)
            subgroups.append((off, slot0, n, s, sp))
            off += n * sp
            slot0 += n
            left -= n
    L = off

    # Supertiles: consecutive subgroups, total width <= ST.
    sts = []          # (base, length, [subgroup indices])
    cur = []
    cur_base = 0
    for gi, (gbase, _, n, s, sp) in enumerate(subgroups):
        w = n * sp
        if cur and (gbase + w - cur_base) > ST:
            sts.append((cur_base, subgroups[gi - 1][0] + subgroups[gi - 1][2] * subgroups[gi - 1][4] - cur_base, cur))
            cur = []
            cur_base = gbase
        cur.append(gi)
    if cur:
        lg = subgroups[cur[-1]]
        sts.append((cur_base, lg[0] + lg[2] * lg[4] - cur_base, cur))

    # DMA chunks: groups of CHUNK_STS supertiles.
    chunks = []       # (base, clen, [st indices])
    for i0 in range(0, len(sts), CHUNK_STS):
        grp = list(range(i0, min(i0 + CHUNK_STS, len(sts))))
        base = sts[grp[0]][0]
        clen = sts[grp[-1]][0] + sts[grp[-1]][1] - base
        chunks.append((base, clen, grp))

    p = _Plan()
    p.template = template
    p.assign = assign
    p.subgroups = subgroups
    p.sts = sts
    p.chunks = chunks
    p.L = L
    p.nslot = len(template)
    p.nslot2 = p.nslot + (p.nslot & 1)
    p.max_clen = max(cl for _, cl, _ in chunks)
    p.halo = 2 * max(n for _, _, n, _, _ in subgroups)
    return p


# --------------------------------------------------------------------------
# Bass program
# --------------------------------------------------------------------------

# weight column-block index in the packed [128, 6*128] conv weight tensor
_BLK = [[0], [1, 2], [3, 4, 5]]


def _build_program(plan: _Plan):
    import concourse.tile as tile
    from concourse import bacc, mybir

    F32 = mybir.dt.float32
    F16 = mybir.dt.float16
    BF16 = mybir.dt.bfloat16
    AF = mybir.ActivationFunctionType
    MAXOP = mybir.AluOpType.max

    nc = bacc.Bacc("TRN2", target_bir_lowering=False, debug=False,
                   num_devices=N_CORES)

    xt_d = nc.dram_tensor("xt", [C, plan.L + plan.halo], BF16, kind="ExternalInput")
    wc_d = nc.dram_tensor("wconv", [C, 6 * C], BF16, kind="ExternalInput")
    lt_d = nc.dram_tensor("lint", [C, 3 * C], F16, kind="ExternalInput")
    bs_d = nc.dram_tensor("biases", [C, 4], F32, kind="ExternalInput")
    out_d = nc.dram_tensor("out", [C, plan.nslot2], F32, kind="ExternalOutput")
    dbg0_d = nc.dram_tensor("dbg0", [C, plan.nslot2], F32, kind="ExternalOutput")
    dbg1_d = nc.dram_tensor("dbg1", [C, plan.nslot2], F16, kind="ExternalOutput")
    dbg2_d = nc.dram_tensor("dbg2", [C, plan.nslot2], F16, kind="ExternalOutput")

    with tile.TileContext(nc) as tc:
        with (
            tc.tile_pool(name="wp", bufs=1) as wp,
            tc.tile_pool(name="xp", bufs=3) as xp,
            tc.tile_pool(name="yp", bufs=3) as yp,
            tc.tile_pool(name="pp", bufs=1) as pp,
            tc.tile_pool(name="ps", bufs=1, space="PSUM") as ps,
        ):
            w_sb = wp.tile([C, 6 * C], BF16, tag="w")
            l_sb = wp.tile([C, 3 * C], F16, tag="l")
            b_sb = wp.tile([C, 4], F32, tag="b")
            nc.sync.dma_start(w_sb[:], wc_d.ap())
            nc.sync.dma_start(l_sb[:], lt_d.ap())
            nc.sync.dma_start(b_sb[:], bs_d.ap())

            pooled0 = pp.tile([C, plan.nslot2], F32, tag="pool0", name="pool0")
            pooled = [None,
                      pp.tile([C, plan.nslot2], F16, tag="pool1", name="pool1"),
                      pp.tile([C, plan.nslot2], F16, tag="pool2", name="pool2")]
            pooled0r = pp.tile([C, plan.nslot2], F16, tag="pool0r", name="pool0r")
            out_sb = pp.tile([C, plan.nslot2], F32, tag="osb", name="osb")
            if plan.nslot2 != plan.nslot:
                nc.vector.memset(pooled0[:, plan.nslot:], 0.0)
                for w in (1, 2):
                    nc.vector.memset(pooled[w][:, plan.nslot:], 0.0)

            for base, clen, st_ids in plan.chunks:
                xc = xp.tile([C, plan.max_clen + plan.halo], BF16, tag="x",
                             name="xc")
                nc.sync.dma_start(
                    xc[:, : clen + plan.halo],
                    xt_d.ap()[:, base : base + clen + plan.halo],
                )
                for sti in st_ids:
                    st_base, st_len, sgs = plan.sts[sti]
                    lo = st_base - base
                    pts = []
                    for w in range(3):
                        pt = ps.tile([C, ST], F32, tag=f"w{w}", name=f"ps{w}")
                        pts.append(pt)
                        for gi in sgs:
                            gbase, slot0, n, s, sp = plan.subgroups[gi]
                            goff = gbase - st_base
                            gw = n * sp
                            a = goff
                            while a < goff + gw:
                                # split at 512-aligned PSUM bank boundaries
                                b = min(goff + gw, (a // MM + 1) * MM)
                                for k in range(w + 1):
                                    nc.tensor.matmul(
                                        pt[:, a:b],
                                        w_sb[:, _BLK[w][k] * C : (_BLK[w][k] + 1) * C],
                                        xc[:, lo + k * n + a : lo + k * n + b],
                                        start=(k == 0),
                                        stop=(k == w),
                                    )
                                a = b
                        if w == 0:
                            # direct strided max reduce from PSUM (exact span)
                            for gi in sgs:
                                gbase, slot0, n, s, sp = plan.subgroups[gi]
                                goff = gbase - st_base
                                src = (
                                    pt[:, goff : goff + n * sp]
                                    .rearrange("p (j i) -> p i j", i=n)[:, :, :s]
                                )
                                nc.vector.tensor_reduce(
                                    out=pooled0[:, slot0 : slot0 + n],
                                    in_=src,
                                    axis=mybir.AxisListType.X,
                                    op=MAXOP,
                                )
                        else:
                            yt = yp.tile([C, ST], F16, tag=f"y{w}", name=f"y{w}")
                            nc.scalar.activation(
                                yt[:, :st_len], pt[:, :st_len], AF.Relu,
                                bias=b_sb[:, w : w + 1],
                            )
                            f1 = yp.tile([C, ST // 2], F16, tag=f"f1_{w}",
                                         name=f"f1_{w}")
                            f2 = yp.tile([C, ST // 4], F16, tag=f"f2_{w}",
                                         name=f"f2_{w}")
                            for gi in sgs:
                                gbase, slot0, n, s, sp = plan.subgroups[gi]
                                goff = gbase - st_base
                                gw = n * sp
                                # zero ragged tail: positions j in [s-w, sp)
                                za = goff + (s - w) * n
                                nc.vector.memset(yt[:, za : goff + gw], 0.0)
                                src = (
                                    yt[:, goff : goff + gw]
                                    .rearrange("p (j i) -> p i j", i=n)
                                )
                                nc.vector.tensor_reduce(
                                    out=pooled[w][:, slot0 : slot0 + n],
                                    in_=src,
                                    axis=mybir.AxisListType.X,
                                    op=MAXOP,
                                )

            nc.scalar.activation(
                pooled0r[:], pooled0[:], AF.Relu, bias=b_sb[:, 0:1]
            )

            for c0 in range(0, plan.nslot2, MM):
                c1 = min(c0 + MM, plan.nslot2)
                lp = ps.tile([C, MM], F32, tag="lin", name="lps")
                srcs = [pooled0r, pooled[1], pooled[2]]
                for w in range(3):
                    nc.tensor.matmul(
                        lp[:, : c1 - c0],
                        l_sb[:, w * C : (w + 1) * C],
                        srcs[w][:, c0:c1],
                        start=(w == 0),
                        stop=(w == 2),
                    )
                nc.scalar.activation(
                    out_sb[:, c0:c1], lp[:, : c1 - c0], AF.Tanh,
                    bias=b_sb[:, 3:4],
                )

            nc.sync.dma_start(out_d.ap(), out_sb[:])
            nc.sync.dma_start(dbg0_d.ap(), pooled0[:])
            nc.sync.dma_start(dbg1_d.ap(), pooled[1][:])
            nc.sync.dma_start(dbg2_d.ap(), pooled[2][:])

    nc.compile()
    return nc


# --------------------------------------------------------------------------
# Host entry point
# --------------------------------------------------------------------------

def kernel(x, sizes, conv_w0, conv_b0, conv_w1, conv_b1, conv_w2, conv_b2,
           lin_w, lin_b):
    global LAST_RESULTS
    from concourse.bass_utils import run_bass_kernel_spmd

    x = np.asarray(x, np.float32)
    sizes = np.asarray(sizes, np.int32)
    convs = [
        (np.asarray(conv_w0, np.float32), np.asarray(conv_b0, np.float32)),
        (np.asarray(conv_w1, np.float32), np.asarray(conv_b1, np.float32)),
        (np.asarray(conv_w2, np.float32), np.asarray(conv_b2, np.float32)),
    ]
    lin_w = np.asarray(lin_w, np.float32)
    lin_b = np.asarray(lin_b, np.float32)

    plan = _build_plan(sizes)
    key = tuple(plan.template)
    if key not in _PROGRAM_CACHE:
        _PROGRAM_CACHE[key] = _build_program(plan)
    nc = _PROGRAM_CACHE[key]

    # Packed conv weights: block b = tap k of stream w, transposed to [C, M].
    wconv = np.empty((C, 6 * C), ml_dtypes.bfloat16)
    for w in range(3):
        cw, _ = convs[w]
        for k in range(w + 1):
            b = _BLK[w][k]
            wconv[:, b * C : (b + 1) * C] = cw[:, :, k].T
    lint = np.empty((C, 3 * C), np.float16)
    for w in range(3):
        lint[:, w * C : (w + 1) * C] = lin_w[:, w * C : (w + 1) * C].T
    biases = np.empty((C, 4), np.float32)
    for w in range(3):
        biases[:, w] = convs[w][1]
    biases[:, 3] = lin_b

    starts = np.cumsum(sizes) - sizes

    in_maps = []
    for c in range(N_CORES):
        amap = np.asarray(plan.assign[c], np.int64)
        col_src = np.full(plan.L + plan.halo, -1, np.int64)
        for gbase, slot0, n, s, sp in plan.subgroups:
            sub = amap[slot0 : slot0 + n]
            real = np.nonzero(sub >= 0)[0]
            if len(real) == 0:
                continue
            # cols[j, i] = gbase + j*n + i ; rows[j, i] = starts[idx] + j
            cols = gbase + np.arange(s)[:, None] * n + real[None, :]
            rows = starts[sub[real]][None, :] + np.arange(s)[:, None]
            col_src[cols.ravel()] = rows.ravel()
        xt = np.zeros((C, plan.L + plan.halo), ml_dtypes.bfloat16)
        valid = col_src >= 0
        xt[:, valid] = x[col_src[valid]].T
        in_maps.append({
            "xt": xt,
            "wconv": wconv,
            "lint": lint,
            "biases": biases,
        })

    res = run_bass_kernel_spmd(nc, in_maps, core_ids=list(range(N_CORES)))
    LAST_RESULTS = res

    out = np.empty((len(sizes), C), np.float32)
    for c in range(N_CORES):
        amap = np.asarray(plan.assign[c], np.int64)
        sel = amap >= 0
        out[amap[sel]] = res.results[c]["out"].T[sel]
    return out


# revision 9
# speedup vs baseline: 1.3680x; 1.3680x over previous
"""Trainium2 Bass kernel for nn_ConvolutionDMax (segment_reduce).

Position-major ("interleaved") layout: slots of equal size s are grouped in
subgroups of n slots; token j of slot i lives at column j*n + i. Conv taps
become matmuls shifted by k*n. Segment max then decomposes into contiguous
tensor_tensor MAX folds (DVE 2x fp16 mode) + one small strided reduce,
instead of a full-rate-limited tensor_reduce over every element (DVE
tensor_reduce runs at 1 elem/cycle regardless of dtype -- measured).

Per supertile (<=1024 PSUM cols, 3 streams):
  - PE: 6 accumulating bf16 matmuls (128x128x<=512).
  - w1/w2: ACT evicts PSUM -> SBUF fp16 with fused Relu+bias (max commutes),
    GPSIMD zeroes the ragged tail blocks (relu'd data >= 0 so zeros are
    neutral), DVE runs 2 contiguous fold halvings + strided reduce tail.
  - w0: DVE strided max reduce direct from PSUM (exact span, raw values).
Final: relu+bias on pooled0, linear via fp16 matmuls, tanh, DMA out.
"""

import os
from collections import defaultdict

import ml_dtypes
import numpy as np

N_CORES = 8
C = 128          # feature dim (partition dim everywhere)
ST = 1024        # supertile positions (2 PSUM banks, fp32)
CHUNK_STS = 4    # supertiles per DMA chunk
MM = 512         # max matmul free dim

_PROGRAM_CACHE = {}
LAST_RESULTS = None  # BassKernelResults of the most recent run (for test.py)


# --------------------------------------------------------------------------
# Layout planning (pure python/numpy, no device deps)
# --------------------------------------------------------------------------

class _Plan:
    __slots__ = (
        "template", "assign", "subgroups", "sts", "chunks", "L", "nslot",
        "nslot2", "max_clen", "halo",
    )


def _build_plan(sizes: np.ndarray) -> _Plan:
    """Template layout shared by all cores + per-core slot assignment.

    subgroups: list of (gbase, slot0, n, s, sp) -- n same-size slots laid out
    position-major in cols [gbase, gbase + n*sp).
    """
    by_size = defaultdict(list)
    for i, s in enumerate(sizes.tolist()):
        by_size[int(s)].append(i)

    template = []                      # slot -> segment size
    assign = [[] for _ in range(N_CORES)]  # core -> slot -> orig idx or -1
    class_counts = []                  # (s, m) in template order
    for s in sorted(by_size, reverse=True):
        idxs = by_size[s]
        m = -(-len(idxs) // N_CORES)
        class_counts.append((s, m))
        for j in range(m):
            template.append(s)
            for c in range(N_CORES):
                k = j * N_CORES + c
                assign[c].append(idxs[k] if k < len(idxs) else -1)

    # Subgroups: per class, chunks of up to nmax slots (nmax even).
    subgroups = []
    off = 0
    slot0 = 0
    for s, m in class_counts:
        sp = (s + 3) & ~3  # mult of 4: fold alignment + even everywhere
        nmax = ST // sp
        if nmax > 1:
            nmax &= ~1
        left = m
        while left > 0:
            n = min(nmax, left)
            subgroups.append((off, slot0, n, s, sp))
            off += n * sp
            slot0 += n
            left -= n
    L = off

    # Supertiles: consecutive subgroups, total width <= ST.
    sts = []          # (base, length, [subgroup indices])
    cur = []
    cur_base = 0
    for gi, (gbase, _, n, s, sp) in enumerate(subgroups):
        w = n * sp
        if cur and (gbase + w - cur_base) > ST:
            sts.append((cur_base, subgroups[gi - 1][0] + subgroups[gi - 1][2] * subgroups[gi - 1][4] - cur_base, cur))
            cur = []
            cur_base = gbase
        cur.append(gi)
    if cur:
        lg = subgroups[cur[-1]]
        sts.append((cur_base, lg[0] + lg[2] * lg[4] - cur_base, cur))

    # DMA chunks: groups of CHUNK_STS supertiles.
    chunks = []       # (base, clen, [st indices])
    for i0 in range(0, len(sts), CHUNK_STS):
        grp = list(range(i0, min(i0 + CHUNK_STS, len(sts))))
        base = sts[grp[0]][0]
        clen = sts[grp[-1]][0] + sts[grp[-1]][1] - base
        chunks.append((base, clen, grp))

    p = _Plan()
    p.template = template
    p.assign = assign
    p.subgroups = subgroups
    p.sts = sts
    p.chunks = chunks
    p.L = L
    p.nslot = len(template)
    p.nslot2 = p.nslot + (p.nslot & 1)
    p.max_clen = max(cl for _, cl, _ in chunks)
    p.halo = 2 * max(n for _, _, n, _, _ in subgroups)
    return p


# --------------------------------------------------------------------------
# Bass program
# --------------------------------------------------------------------------

# weight column-block index in the packed [128, 6*128] conv weight tensor
_BLK = [[0], [1, 2], [3, 4, 5]]


def _build_program(plan: _Plan):
    import concourse.tile as tile
    from concourse import bacc, mybir

    F32 = mybir.dt.float32
    F16 = mybir.dt.float16
    BF16 = mybir.dt.bfloat16
    AF = mybir.ActivationFunctionType
    MAXOP = mybir.AluOpType.max

    nc = bacc.Bacc("TRN2", target_bir_lowering=False, debug=False,
                   num_devices=N_CORES)

    xt_d = nc.dram_tensor("xt", [C, plan.L + plan.halo], BF16, kind="ExternalInput")
    wc_d = nc.dram_tensor("wconv", [C, 6 * C], BF16, kind="ExternalInput")
    lt_d = nc.dram_tensor("lint", [C, 3 * C], F16, kind="ExternalInput")
    bs_d = nc.dram_tensor("biases", [C, 4], F32, kind="ExternalInput")
    out_d = nc.dram_tensor("out", [C, plan.nslot2], F32, kind="ExternalOutput")

    with tile.TileContext(nc) as tc:
        with (
            tc.tile_pool(name="wp", bufs=1) as wp,
            tc.tile_pool(name="xp", bufs=3) as xp,
            tc.tile_pool(name="yp", bufs=3) as yp,
            tc.tile_pool(name="pp", bufs=1) as pp,
            tc.tile_pool(name="ps", bufs=1, space="PSUM") as ps,
        ):
            w_sb = wp.tile([C, 6 * C], BF16, tag="w")
            l_sb = wp.tile([C, 3 * C], F16, tag="l")
            b_sb = wp.tile([C, 4], F32, tag="b")
            nc.sync.dma_start(w_sb[:], wc_d.ap())
            nc.sync.dma_start(l_sb[:], lt_d.ap())
            nc.sync.dma_start(b_sb[:], bs_d.ap())

            pooled0 = pp.tile([C, plan.nslot2], F32, tag="pool0", name="pool0")
            pooled = [None,
                      pp.tile([C, plan.nslot2], F16, tag="pool1", name="pool1"),
                      pp.tile([C, plan.nslot2], F16, tag="pool2", name="pool2")]
            pooled0r = pp.tile([C, plan.nslot2], F16, tag="pool0r", name="pool0r")
            out_sb = pp.tile([C, plan.nslot2], F32, tag="osb", name="osb")
            if plan.nslot2 != plan.nslot:
                nc.vector.memset(pooled0[:, plan.nslot:], 0.0)
                for w in (1, 2):
                    nc.vector.memset(pooled[w][:, plan.nslot:], 0.0)

            for base, clen, st_ids in plan.chunks:
                xc = xp.tile([C, plan.max_clen + plan.halo], BF16, tag="x",
                             name="xc")
                nc.sync.dma_start(
                    xc[:, : clen + plan.halo],
                    xt_d.ap()[:, base : base + clen + plan.halo],
                )
                for sti in st_ids:
                    st_base, st_len, sgs = plan.sts[sti]
                    lo = st_base - base
                    pts = []
                    for w in range(3):
                        pt = ps.tile([C, ST], F32, tag=f"w{w}", name=f"ps{w}")
                        pts.append(pt)
                        for gi in sgs:
                            gbase, slot0, n, s, sp = plan.subgroups[gi]
                            goff = gbase - st_base
                            gw = n * sp
                            a = goff
                            while a < goff + gw:
                                # split at 512-aligned PSUM bank boundaries
                                b = min(goff + gw, (a // MM + 1) * MM)
                                for k in range(w + 1):
                                    nc.tensor.matmul(
                                        pt[:, a:b],
                                        w_sb[:, _BLK[w][k] * C : (_BLK[w][k] + 1) * C],
                                        xc[:, lo + k * n + a : lo + k * n + b],
                                        start=(k == 0),
                                        stop=(k == w),
                                    )
                                a = b
                        if w == 0:
                            # direct strided max reduce from PSUM (exact span)
                            for gi in sgs:
                                gbase, slot0, n, s, sp = plan.subgroups[gi]
                                goff = gbase - st_base
                                src = (
                                    pt[:, goff : goff + n * sp]
                                    .rearrange("p (j i) -> p i j", i=n)[:, :, :s]
                                )
                                nc.vector.tensor_reduce(
                                    out=pooled0[:, slot0 : slot0 + n],
                                    in_=src,
                                    axis=mybir.AxisListType.X,
                                    op=MAXOP,
                                )
                        else:
                            yt = yp.tile([C, ST], F16, tag=f"y{w}", name=f"y{w}")
                            nc.scalar.activation(
                                yt[:, :st_len], pt[:, :st_len], AF.Relu,
                                bias=b_sb[:, w : w + 1],
                            )
                            f1 = yp.tile([C, ST // 2], F16, tag=f"f1_{w}",
                                         name=f"f1_{w}")
                            f2 = yp.tile([C, ST // 4], F16, tag=f"f2_{w}",
                                         name=f"f2_{w}")
                            for gi in sgs:
                                gbase, slot0, n, s, sp = plan.subgroups[gi]
                                goff = gbase - st_base
                                gw = n * sp
                                # zero ragged tail: positions j in [s-w, sp)
                                za = goff + (s - w) * n
                                nc.gpsimd.memset(yt[:, za : goff + gw], 0.0)
                                h1 = gw // 2
                                nc.vector.tensor_tensor(
                                    out=f1[:, goff // 2 : goff // 2 + h1],
                                    in0=yt[:, goff : goff + h1],
                                    in1=yt[:, goff + h1 : goff + gw],
                                    op=MAXOP,
                                )
                                h2 = gw // 4
                                nc.vector.tensor_tensor(
                                    out=f2[:, goff // 4 : goff // 4 + h2],
                                    in0=f1[:, goff // 2 : goff // 2 + h2],
                                    in1=f1[:, goff // 2 + h2 : goff // 2 + h1],
                                    op=MAXOP,
                                )
                                src = (
                                    f2[:, goff // 4 : goff // 4 + h2]
                                    .rearrange("p (j i) -> p i j", i=n)
                                )
                                nc.vector.tensor_reduce(
                                    out=pooled[w][:, slot0 : slot0 + n],
                                    in_=src,
                                    axis=mybir.AxisListType.X,
                                    op=MAXOP,
                                )

            nc.scalar.activation(
                pooled0r[:], pooled0[:], AF.Relu, bias=b_sb[:, 0:1]
            )

            for c0 in range(0, plan.nslot2, MM):
                c1 = min(c0 + MM, plan.nslot2)
                lp = ps.tile([C, MM], F32, tag="lin", name="lps")
                srcs = [pooled0r, pooled[1], pooled[2]]
                for w in range(3):
                    nc.tensor.matmul(
                        lp[:, : c1 - c0],
                        l_sb[:, w * C : (w + 1) * C],
                        srcs[w][:, c0:c1],
                        start=(w == 0),
                        stop=(w == 2),
                    )
                nc.scalar.activation(
                    out_sb[:, c0:c1], lp[:, : c1 - c0], AF.Tanh,
                    bias=b_sb[:, 3:4],
                )

            nc.sync.dma_start(out_d.ap(), out_sb[:])

    nc.compile()
    return nc


# --------------------------------------------------------------------------
# Host entry point
# --------------------------------------------------------------------------

def kernel(x, sizes, conv_w0, conv_b0, conv_w1, conv_b1, conv_w2, conv_b2,
           lin_w, lin_b):
    global LAST_RESULTS
    from concourse.bass_utils import run_bass_kernel_spmd

    x = np.asarray(x, np.float32)
    sizes = np.asarray(sizes, np.int32)
    convs = [
        (np.asarray(conv_w0, np.float32), np.asarray(conv_b0, np.float32)),
        (np.asarray(conv_w1, np.float32), np.asarray(conv_b1, np.float32)),
        (np.asarray(conv_w2, np.float32), np.asarray(conv_b2, np.float32)),
    ]
    lin_w = np.asarray(lin_w, np.float32)
    lin_b = np.asarray(lin_b, np.float32)

    plan = _build_plan(sizes)
    key = tuple(plan.template)
    if key not in _PROGRAM_CACHE:
        _PROGRAM_CACHE[key] = _build_program(plan)
    nc = _PROGRAM_CACHE[key]

    # Packed conv weights: block b = tap k of stream w, transposed to [C, M].
    wconv = np.empty((C, 6 * C), ml_dtypes.bfloat16)
    for w in range(3):
        cw, _ = convs[w]
        for k in range(w + 1):
            b = _BLK[w][k]
            wconv[:, b * C : (b + 1) * C] = cw[:, :, k].T
    lint = np.empty((C, 3 * C), np.float16)
    for w in range(3):
        lint[:, w * C : (w + 1) * C] = lin_w[:, w * C : (w + 1) * C].T
    biases = np.empty((C, 4), np.float32)
    for w in range(3):
        biases[:, w] = convs[w][1]
    biases[:, 3] = lin_b

    starts = np.cumsum(sizes) - sizes

    in_maps = []
    for c in range(N_CORES):
        amap = np.asarray(plan.assign[c], np.int64)
        col_src = np.full(plan.L + plan.halo, -1, np.int64)
        for gbase, slot0, n, s, sp in plan.subgroups:
            sub = amap[slot0 : slot0 + n]
            real = np.nonzero(sub >= 0)[0]
            if len(real) == 0:
                continue
            # cols[j, i] = gbase + j*n + i ; rows[j, i] = starts[idx] + j
            cols = gbase + np.arange(s)[:, None] * n + real[None, :]
            rows = starts[sub[real]][None, :] + np.arange(s)[:, None]
            col_src[cols.ravel()] = rows.ravel()
        xt = np.zeros((C, plan.L + plan.halo), ml_dtypes.bfloat16)
        valid = col_src >= 0
        xt[:, valid] = x[col_src[valid]].T
        in_maps.append({
            "xt": xt,
            "wconv": wconv,
            "lint": lint,
            "biases": biases,
        })

    res = run_bass_kernel_spmd(nc, in_maps, core_ids=list(range(N_CORES)))
    LAST_RESULTS = res

    out = np.empty((len(sizes), C), np.float32)
    for c in range(N_CORES):
        amap = np.asarray(plan.assign[c], np.int64)
        sel = amap >= 0
        out[amap[sel]] = res.results[c]["out"].T[sel]
    return out
